# revision 13
# baseline (speedup 1.0000x reference)
"""CoNHD GD-layer Trainium2 kernel (8-core SPMD, Bass/Tile), v6.

Math (see the reference): two independent set-attention stacks over
fixed-size mailbox groups (v-side: N=2048 nodes x DV=32, e-side: M=4096
hyperedges x DE=16), followed by a 4*D -> D update linear applied in two
eid orders.

Device strategy (engine-balanced block-diagonal attention, 3-way
software pipelining):
  - Shard rows (E=65536) across 8 cores; group attention never crosses
    the per-core boundary.  Activations are bf16 on chip; PSUM accum fp32.
  - Tiles from both sides are processed in stage-interleaved groups of 3:
    independent dependency chains keep every engine's in-order queue fed
    while any one tile waits on a cross-engine dependency.
  - Scores are computed only for the two diagonal 128x128 key x query
    blocks of each 256-row tile (groups of 32/16 never cross them), all
    4 heads sharing one [128,512] PSUM bank; the block-diagonal group
    mask is a rank-G accumulating matmul (exp(x/16 - C) removes 16C).
  - AV is computed transposed (queries on partitions); ap=1 ones-matmuls
    append the softmax denominators as a 65th pO column, so normalization
    is one broadcast tensor_tensor per block (queries are partitions).
  - Normalized O is transposed back to feature-major on the PE (identity
    matmul) and +Qh is a single fused add.
  - bk provably cancels in softmax and is dropped.  When all biases are
    zero (the spec fills them with zeros) the fast path fuses every
    PSUM evacuation into one [128,(2,256)] op; otherwise a general path
    adds per-chunk per-partition biases.
  - The Activation engine's queue carries only critical-path work (exp,
    split per head-pair so AV starts early) plus post-score ops.
    Evacuations are spread across DVE / GPSIMD.
  - The update linear runs feature-major (out [of, rows]); outputs land
    transposed in HBM ([128, 2, R] bf16) and the host re-assembles +
    combines the two eid orders:
      out_in  = A + P3[inv_perm],  out_con = out_in[perm].
  - Host-side layouts make every per-tile DMA a single [128,2,256]
    transfer; weights load as a handful of blob DMAs (the SP sequencer's
    ~650ns per-DMA issue cost would otherwise serialize).

kernel(**inputs) takes the full unsharded inputs and returns [2, E, D] f32.
"""
import sys

if "/opt/trn_rl_repo" not in sys.path:
    sys.path.insert(0, "/opt/trn_rl_repo")

from contextlib import ExitStack

import numpy as np

import concourse.mybir as mybir
import concourse.tile as tile
from concourse import bacc
from concourse.bass_utils import run_bass_kernel_spmd

F32 = mybir.dt.float32
BF16 = mybir.dt.bfloat16
AF = mybir.ActivationFunctionType
ALU = mybir.AluOpType

N, DV, M, DE, E = 2048, 32, 4096, 16, 65536
D, WD, L, H = 256, 64, 2, 4
NCORES = 8
MASK_C = 30.0
PS_B = 8   # 2KB PSUM slots (banks) in the main pool
GRP = 3    # tiles processed stage-interleaved
SBUF_B = 3 * GRP


def _sab_layer(nc, pools, items, zb, ident, negc, ones1):
    """One SAB layer on a group of stage-interleaved 256-row tiles.

    items: list of (Xt, C); Xt is a [128, 2, 256] bf16 tile (feature-major,
    fb chunks).  Returns the new (Xt, C) list.
    """
    sb, ps = pools
    n = len(items)

    # --- Q/K projections (feature-major) ---
    psQ, psK = [], []
    for i, (Xt, C) in enumerate(items):
        W = C["W"]
        q = ps.tile([128, 2, 256], F32, tag="bank", name="psQ", bufs=PS_B)
        k = ps.tile([128, 2, 256], F32, tag="bank", name="psK", bufs=PS_B)
        for fb in range(2):
            for kb in range(2):
                nc.tensor.matmul(q[:, fb, :], W("q", kb, fb),
                                 Xt[:, kb, :], start=(kb == 0), stop=(kb == 1))
                nc.tensor.matmul(k[:, fb, :], W("k", kb, fb),
                                 Xt[:, kb, :], start=(kb == 0), stop=(kb == 1))
        psQ.append(q)
        psK.append(k)
    Qt = [sb.tile([128, 2, 256], BF16, tag="Qt", name="Qt", bufs=SBUF_B)
          for _ in range(n)]
    Kt = [sb.tile([128, 2, 256], BF16, tag="Kt", name="Kt", bufs=SBUF_B)
          for _ in range(n)]
    for i, (Xt, C) in enumerate(items):
        if zb:
            nc.vector.tensor_copy(Qt[i][:], psQ[i][:])
        else:
            Bq = C["Bq"]
            for fb in range(2):
                nc.vector.tensor_scalar_add(Qt[i][:, fb, :], psQ[i][:, fb, :],
                                            Bq[:, fb:fb + 1])
        nc.scalar.copy(Kt[i][:], psK[i][:])

    # --- V projection (row-major) ---
    psV = []
    for i, (Xt, C) in enumerate(items):
        W = C["W"]
        v = ps.tile([128, 2, 4, 64], F32, tag="bank", name="psV", bufs=PS_B)
        for rb in range(2):
            for kb in range(2):
                nc.tensor.matmul(v[:, rb, :, :],
                                 Xt[:, kb, rb * 128:(rb + 1) * 128],
                                 W("v", kb, None), start=(kb == 0), stop=(kb == 1))
        psV.append(v)
    Vr = [sb.tile([128, 2, 4, 64], BF16, tag="Vr", name="Vr", bufs=2 * GRP)
          for _ in range(n)]
    for i in range(n):
        nc.scalar.copy(Vr[i][:], psV[i][:])

    # --- attention per diagonal 128x128 block ---
    O_rm = [sb.tile([128, 2, 4, 64], BF16, tag="Orm", name="O_rm",
                    bufs=2 * GRP) for _ in range(n)]
    for b in range(2):
        psS = []
        for i, (Xt, C) in enumerate(items):
            s = ps.tile([128, 512], F32, tag="bank", name="psS", bufs=PS_B)
            for h in range(H):
                p, off = h // 2, (h % 2) * 64
                bs = slice(b * 128, (b + 1) * 128)
                nc.tensor.matmul(s[:, h * 128:(h + 1) * 128],
                                 C["Gk"][:], C["Gq"][:], start=True, stop=False)
                nc.tensor.matmul(s[:, h * 128:(h + 1) * 128],
                                 Kt[i][off:off + 64, p, bs],
                                 Qt[i][off:off + 64, p, bs],
                                 start=False, stop=True)
            psS.append(s)
        eS = []
        for i in range(n):
            e = sb.tile([128, 512], BF16, tag="eS", name="eS", bufs=2 * GRP)
            nc.scalar.activation(e[:], psS[i][:], AF.Exp, bias=negc[:],
                                 scale=1.0 / 16.0)
            eS.append(e)
        for i in range(n):
            pO = ps.tile([128, 4, 65], F32, tag="bank", name="pO", bufs=PS_B)
            for h in range(H):
                nc.tensor.matmul(pO[:, h, 0:64], eS[i][:, h * 128:(h + 1) * 128],
                                 Vr[i][:, b, h, :], start=True, stop=True)
                nc.tensor.matmul(pO[:, h, 64:65],
                                 eS[i][:, h * 128:(h + 1) * 128],
                                 ones1[:], start=True, stop=True)
            rec = sb.tile([128, 4, 1], F32, tag="rec", name="rec", bufs=2 * GRP)
            nc.vector.reciprocal(rec[:], pO[:, :, 64:65])
            nc.vector.tensor_tensor(O_rm[i][:, b, :, :], pO[:, :, 0:64],
                                    rec[:].broadcast_to([128, 4, 64]), ALU.mult)

    # --- transpose O back to feature-major, + Qh ---
    Ot = [sb.tile([128, 2, 256], BF16, tag="Ot", name="Ot", bufs=SBUF_B)
          for _ in range(n)]
    for i, (Xt, C) in enumerate(items):
        psOT = ps.tile([128, 2, 256], BF16, tag="bank", name="psOT", bufs=PS_B)
        for b in range(2):
            for fb in range(2):
                nc.tensor.transpose(psOT[:, fb, b * 128:(b + 1) * 128],
                                    O_rm[i][:, b, fb * 2:fb * 2 + 2, :],
                                    ident[:])
        if zb:
            nc.vector.tensor_add(Ot[i][:], psOT[:], Qt[i][:])
        else:
            Bv = C["Bv"]
            for fb in range(2):
                nc.vector.scalar_tensor_tensor(
                    Ot[i][:, fb, :], psOT[:, fb, :],
                    Bv[:, fb:fb + 1], Qt[i][:, fb, :], ALU.add, ALU.add)

    # --- Z = O + relu(O @ Wo + bo) ---
    psR = []
    for i, (Xt, C) in enumerate(items):
        W = C["W"]
        r = ps.tile([128, 2, 256], F32, tag="bank", name="psR", bufs=PS_B)
        for fb in range(2):
            for kb in range(2):
                nc.tensor.matmul(r[:, fb, :], W("o", kb, fb),
                                 Ot[i][:, kb, :], start=(kb == 0), stop=(kb == 1))
        psR.append(r)
    out = []
    for i, (Xt, C) in enumerate(items):
        Rt = sb.tile([128, 2, 256], BF16, tag="Rt", name="Rt", bufs=2 * GRP)
        if zb:
            nc.scalar.activation(Rt[:], psR[i][:], AF.Relu)
        else:
            Bo = C["Bo"]
            for fb in range(2):
                nc.scalar.activation(Rt[:, fb, :], psR[i][:, fb, :],
                                     AF.Relu, bias=Bo[:, fb:fb + 1])
        Zt = sb.tile([128, 2, 256], BF16, tag="Zt", name="Zt", bufs=SBUF_B)
        nc.gpsimd.tensor_add(Zt[:], Ot[i][:], Rt[:])
        out.append((Zt, C))
    return out


def build_program(R, zb):
    """Per-core SPMD program; R = rows per core; zb = all biases zero."""
    NT = R // 256
    nc = bacc.Bacc("TRN2", target_bir_lowering=False, debug=False)

    def din(name, shape, dt=BF16):
        return nc.dram_tensor(name, shape, dt, kind="ExternalInput").ap()

    xvt_d = din("xvt", [128, 2, R])
    wvt_d = din("wvt", [WD, R])
    xet_d = din("xet", [128, 2, R])
    wet_d = din("wet", [WD, R])
    x0t_d = din("x0t", [128, 2, R])
    peW_v_d = din("peW_v", [WD, D])
    peW_e_d = din("peW_e", [WD, D])
    Wv_d = din("W_v", [128, L, 4, 2, 256])
    We_d = din("W_e", [128, L, 4, 2, 256])
    Wu_d = din("W_upd", [128, 4, 2, 256])
    # bias blob [128, 32] f32: cols 0..3 Bq_v(l,fb), 4..7 Bo_v, 8..11 Bv_v,
    # 12..23 same for e, 24..25 peb_v, 26..27 peb_e, 28..29 updb
    bias_d = din("biases", [128, 32], F32)
    Gk_v_d = din("Gk_v", [4, 128])
    Gq_v_d = din("Gq_v", [4, 128])
    Gk_e_d = din("Gk_e", [8, 128])
    Gq_e_d = din("Gq_e", [8, 128])
    ident_d = din("ident", [128, 128])

    At_d = nc.dram_tensor("At", [128, 2, R], BF16, kind="ExternalOutput").ap()
    P3t_d = nc.dram_tensor("P3t", [128, 2, R], BF16, kind="ExternalOutput").ap()

    with tile.TileContext(nc) as tc, ExitStack() as es, \
            nc.allow_low_precision(reason="bf16 activations, fp32 PSUM accum"):
        const = es.enter_context(tc.tile_pool(name="const", bufs=1))
        sb = es.enter_context(tc.tile_pool(name="sb", bufs=4))
        inp = es.enter_context(tc.tile_pool(name="inp", bufs=3 * GRP))
        outp = es.enter_context(tc.tile_pool(name="outp", bufs=2 * GRP))
        ps = es.enter_context(tc.tile_pool(name="ps", bufs=1, space="PSUM"))
        pools = (sb, ps)

        negc = const.tile([128, 1], F32, tag="negc", name="negc")
        nc.vector.memset(negc[:], -MASK_C)
        ones1 = const.tile([128, 1], BF16, tag="ones1", name="ones1")
        nc.vector.memset(ones1[:], 1.0)
        ident = const.tile([128, 128], BF16, tag="ident", name="ident")
        nc.sync.dma_start(ident[:], ident_d)

        bias = const.tile([128, 32], F32, tag="bias", name="bias")
        nc.sync.dma_start(bias[:], bias_d)

        peW = {}
        for s, peW_d in (("v", peW_v_d), ("e", peW_e_d)):
            t = const.tile([WD, D], BF16, tag=f"peW_{s}", name=f"peW_{s}")
            nc.sync.dma_start(t[:], peW_d)
            peW[s] = t
        peb = {"v": bias[:, 24:26], "e": bias[:, 26:28]}
        updb = bias[:, 28:30]

        Wall, G = {}, {}
        for s, W_d in (("v", Wv_d), ("e", We_d)):
            t = const.tile([128, L, 4, 2, 256], BF16, tag=f"W_{s}", name=f"W_{s}")
            nc.sync.dma_start(t[:], W_d)
            Wall[s] = t
        for s, gk_d, gq_d, g in (("v", Gk_v_d, Gq_v_d, 4), ("e", Gk_e_d, Gq_e_d, 8)):
            gk = const.tile([g, 128], BF16, tag=f"Gk{s}", name=f"Gk{s}")
            nc.sync.dma_start(gk[:], gk_d)
            gq = const.tile([g, 128], BF16, tag=f"Gq{s}", name=f"Gq{s}")
            nc.sync.dma_start(gq[:], gq_d)
            G[s] = (gk, gq)
        Wu = const.tile([128, 4, 2, 256], BF16, tag="Wu", name="Wu")
        nc.sync.dma_start(Wu[:], Wu_d)

        PI = {"q": 0, "k": 1, "v": 2, "o": 3}

        def mkW(s, l):
            def W(p, kb, fb):
                if fb is None:
                    return Wall[s][:, l, PI[p], kb, :]
                return Wall[s][:, l, PI[p], kb, fb * 128:(fb + 1) * 128]
            return W

        side_consts = {}
        for s in ("v", "e"):
            off = 0 if s == "v" else 12
            side_consts[s] = [{
                "W": mkW(s, l),
                "Bq": bias[:, off + l * 2:off + l * 2 + 2],
                "Bo": bias[:, off + 4 + l * 2:off + 4 + l * 2 + 2],
                "Bv": bias[:, off + 8 + l * 2:off + 8 + l * 2 + 2],
                "Gk": G[s][0], "Gq": G[s][1],
            } for l in range(L)]

        side_io = {"v": (xvt_d, wvt_d, At_d), "e": (xet_d, wet_d, P3t_d)}
        tiles = [(s, t) for t in range(NT) for s in ("v", "e")]
        groups = [tiles[g:g + GRP] for g in range(0, len(tiles), GRP)]

        for grp in groups:
            # --- loads ---
            xts, wts, x0s = [], [], {}
            for s, t in grp:
                cs = slice(t * 256, (t + 1) * 256)
                xt_d, wt_d, _ = side_io[s]
                xt = inp.tile([128, 2, 256], BF16, tag="xt", name="xt")
                nc.sync.dma_start(xt[:], xt_d[:, :, cs])
                xts.append(xt)
                wt = inp.tile([WD, 256], BF16, tag="wt", name="wt")
                nc.sync.dma_start(wt[:], wt_d[:, cs])
                wts.append(wt)
                if s == "v" and t not in x0s:
                    x0 = inp.tile([128, 2, 256], BF16, tag="x0", name="x0",
                                  bufs=GRP)
                    nc.sync.dma_start(x0[:], x0t_d[:, :, cs])
                    x0s[t] = x0

            # --- mailbox prep: Xt = xt + peW^T wt (+ peb) ---
            psP, Xts = [], []
            for gi, (s, t) in enumerate(grp):
                p = ps.tile([128, 2, 256], F32, tag="bank", name="psP",
                            bufs=PS_B)
                for fb in range(2):
                    nc.tensor.matmul(p[:, fb, :],
                                     peW[s][:, fb * 128:(fb + 1) * 128],
                                     wts[gi][:], start=True, stop=True)
                psP.append(p)
            for gi, (s, t) in enumerate(grp):
                Xt = sb.tile([128, 2, 256], BF16, tag="Xt", name="Xt",
                             bufs=SBUF_B)
                if zb:
                    nc.vector.tensor_add(Xt[:], psP[gi][:], xts[gi][:])
                else:
                    for fb in range(2):
                        nc.vector.scalar_tensor_tensor(
                            Xt[:, fb, :], psP[gi][:, fb, :],
                            peb[s][:, fb:fb + 1], xts[gi][:, fb, :],
                            ALU.add, ALU.add)
                Xts.append(Xt)

            # --- 2 SAB layers, stage-interleaved across the group ---
            items = [(Xts[gi], side_consts[s][0]) for gi, (s, t) in enumerate(grp)]
            items = _sab_layer(nc, pools, items, zb, ident, negc, ones1)
            items = [(items[gi][0], side_consts[s][1])
                     for gi, (s, t) in enumerate(grp)]
            items = _sab_layer(nc, pools, items, zb, ident, negc, ones1)

            # --- update linear, feature-major: out[of, rows] ---
            psA = []
            for gi, (s, t) in enumerate(grp):
                Zt = items[gi][0]
                a = ps.tile([128, 2, 256], F32, tag="bank", name="psA",
                            bufs=PS_B)
                srcs = (((xts[gi], 0), (Zt, 1), (x0s[t], 3)) if s == "v"
                        else ((Zt, 2),))
                for of in range(2):
                    first = True
                    for src, j in srcs:
                        for kb in range(2):
                            nc.tensor.matmul(
                                a[:, of, :],
                                Wu[:, j, kb, of * 128:(of + 1) * 128],
                                src[:, kb, :], start=first,
                                stop=(src is srcs[-1][0] and kb == 1))
                            first = False
                psA.append(a)
            for gi, (s, t) in enumerate(grp):
                cs = slice(t * 256, (t + 1) * 256)
                out_d = side_io[s][2]
                Ao = outp.tile([128, 2, 256], BF16, tag="Ao", name="Ao")
                if zb or s == "e":
                    nc.scalar.copy(Ao[:], psA[gi][:])
                else:
                    for of in range(2):
                        nc.scalar.activation(Ao[:, of, :], psA[gi][:, of, :],
                                             AF.Identity,
                                             bias=updb[:, of:of + 1])
                nc.sync.dma_start(out_d[:, :, cs], Ao[:])

    nc.compile()
    return nc


def _make_group_consts(n_group):
    """Gk [G,128] (bf16, 16C on own group) and Gq4 [G,512] (1 on own group,
    tiled for 4 heads)."""
    G = 128 // n_group
    Gk = np.zeros((G, 128), np.float32)
    Gq = np.zeros((G, 128), np.float32)
    for g in range(G):
        Gk[g, g * n_group:(g + 1) * n_group] = 16.0 * MASK_C
        Gq[g, g * n_group:(g + 1) * n_group] = 1.0
    return Gk, Gq


_PROGRAM_CACHE = {}


def _get_program(R, zb=True):
    key = (R, zb)
    if key not in _PROGRAM_CACHE:
        _PROGRAM_CACHE[key] = build_program(R, zb)
    return _PROGRAM_CACHE[key]


def kernel(co_feat_in, co_feat_con, co_feat_0, weight_in, weight_con,
           pe_v_W, pe_v_b, pe_e_W, pe_e_b,
           Wq_v, bq_v, Wk_v, bk_v, Wv_v, bv_v, Wo_v, bo_v,
           Wq_e, bq_e, Wk_e, bk_e, Wv_e, bv_e, Wo_e, bo_e,
           upd_W, upd_b, perm):
    bf = np.dtype(mybir.dt.np(BF16))
    f = lambda x: np.asarray(x, np.float32)
    perm = np.asarray(perm)

    R = E // NCORES
    zb = all(not np.any(f(b)) for b in
             (bq_v, bv_v, bo_v, bq_e, bv_e, bo_e, pe_v_b, pe_e_b, upd_b))
    nc = _get_program(R, zb)

    Gk_v, Gq_v = _make_group_consts(DV)
    Gk_e, Gq_e = _make_group_consts(DE)

    # bias blob [128, 32]
    bias = np.zeros((128, 32), np.float32)
    for off, (bq, bo, bv) in ((0, (bq_v, bo_v, bv_v)), (12, (bq_e, bo_e, bv_e))):
        for l in range(L):
            for fb in range(2):
                bias[:, off + l * 2 + fb] = f(bq)[l, fb * 128:(fb + 1) * 128]
                bias[:, off + 4 + l * 2 + fb] = f(bo)[l, fb * 128:(fb + 1) * 128]
                bias[:, off + 8 + l * 2 + fb] = f(bv)[l, fb * 128:(fb + 1) * 128]
    for fb in range(2):
        bias[:, 24 + fb] = f(pe_v_b)[fb * 128:(fb + 1) * 128]
        bias[:, 26 + fb] = f(pe_e_b)[fb * 128:(fb + 1) * 128]
        bias[:, 28 + fb] = f(upd_b)[fb * 128:(fb + 1) * 128]

    def wblob(Wq, Wk, Wv, Wo):
        # [L,4,D,D] -> [128, L, 4, 2, 256]
        Ws = np.stack([f(Wq), f(Wk), f(Wv), f(Wo)], axis=1)  # [L,4,D,D]
        return np.ascontiguousarray(
            Ws.reshape(L, 4, 2, 128, 256).transpose(3, 0, 1, 2, 4)).astype(bf)

    Wu = np.ascontiguousarray(
        f(upd_W).reshape(4, 2, 128, 256).transpose(2, 0, 1, 3)).astype(bf)

    def xlayout(x, rs):
        # [R, 256] slice -> [128, 2, R]
        a = np.asarray(x)[rs]
        return np.ascontiguousarray(
            a.reshape(a.shape[0], 2, 128).transpose(2, 1, 0)).astype(bf)

    shared = {
        "peW_v": f(pe_v_W).astype(bf), "peW_e": f(pe_e_W).astype(bf),
        "W_v": wblob(Wq_v, Wk_v, Wv_v, Wo_v),
        "W_e": wblob(Wq_e, Wk_e, Wv_e, Wo_e),
        "W_upd": Wu,
        "biases": bias,
        "Gk_v": Gk_v.astype(bf), "Gq_v": Gq_v.astype(bf),
        "Gk_e": Gk_e.astype(bf), "Gq_e": Gq_e.astype(bf),
        "ident": np.eye(128, dtype=np.float32).astype(bf),
    }

    in_maps = []
    for c in range(NCORES):
        rs = slice(c * R, (c + 1) * R)
        m = dict(shared)
        m["xvt"] = xlayout(co_feat_in, rs)
        m["xet"] = xlayout(co_feat_con, rs)
        m["x0t"] = xlayout(co_feat_0, rs)
        m["wvt"] = np.asarray(weight_in)[rs].T.astype(bf)
        m["wet"] = np.asarray(weight_con)[rs].T.astype(bf)
        in_maps.append(m)

    res = run_bass_kernel_spmd(nc, in_maps, core_ids=list(range(NCORES)))

    def unlayout(o):
        # [128, 2, R] -> [R, 256] f32
        return np.asarray(o).transpose(2, 1, 0).reshape(R, 256).astype(np.float32)

    A = np.concatenate([unlayout(res.results[c]["At"]) for c in range(NCORES)])
    P3 = np.concatenate([unlayout(res.results[c]["P3t"]) for c in range(NCORES)])

    inv_perm = np.argsort(perm)
    out_in = A + P3[inv_perm]
    return np.stack([out_in, out_in[perm]]).astype(np.float32)


# revision 15
# speedup vs baseline: 1.3393x; 1.3393x over previous
"""CoNHD GD-layer Trainium2 kernel (8-core SPMD, Bass/Tile), v6.

Math (see the reference): two independent set-attention stacks over
fixed-size mailbox groups (v-side: N=2048 nodes x DV=32, e-side: M=4096
hyperedges x DE=16), followed by a 4*D -> D update linear applied in two
eid orders.

Device strategy (engine-balanced block-diagonal attention, 3-way
software pipelining):
  - Shard rows (E=65536) across 8 cores; group attention never crosses
    the per-core boundary.  Activations are bf16 on chip; PSUM accum fp32.
  - Tiles from both sides are processed in stage-interleaved groups of 3:
    independent dependency chains keep every engine's in-order queue fed
    while any one tile waits on a cross-engine dependency.
  - Scores are computed only for the two diagonal 128x128 key x query
    blocks of each 256-row tile (groups of 32/16 never cross them), all
    4 heads sharing one [128,512] PSUM bank; the block-diagonal group
    mask is a rank-G accumulating matmul (exp(x/16 - C) removes 16C).
  - AV is computed transposed (queries on partitions); ap=1 ones-matmuls
    append the softmax denominators as a 65th pO column, so normalization
    is one broadcast tensor_tensor per block (queries are partitions).
  - Normalized O is transposed back to feature-major on the PE (identity
    matmul) and +Qh is a single fused add.
  - bk provably cancels in softmax and is dropped.  When all biases are
    zero (the spec fills them with zeros) the fast path fuses every
    PSUM evacuation into one [128,(2,256)] op; otherwise a general path
    adds per-chunk per-partition biases.
  - The Activation engine's queue carries only critical-path work (exp,
    split per head-pair so AV starts early) plus post-score ops.
    Evacuations are spread across DVE / GPSIMD.
  - The update linear runs feature-major (out [of, rows]); outputs land
    transposed in HBM ([128, 2, R] bf16) and the host re-assembles +
    combines the two eid orders:
      out_in  = A + P3[inv_perm],  out_con = out_in[perm].
  - Host-side layouts make every per-tile DMA a single [128,2,256]
    transfer; weights load as a handful of blob DMAs (the SP sequencer's
    ~650ns per-DMA issue cost would otherwise serialize).

kernel(**inputs) takes the full unsharded inputs and returns [2, E, D] f32.
"""
import sys

if "/opt/trn_rl_repo" not in sys.path:
    sys.path.insert(0, "/opt/trn_rl_repo")

from contextlib import ExitStack

import numpy as np

import concourse.mybir as mybir
import concourse.tile as tile
from concourse import bacc
from concourse.bass_utils import run_bass_kernel_spmd

F32 = mybir.dt.float32
BF16 = mybir.dt.bfloat16
AF = mybir.ActivationFunctionType
ALU = mybir.AluOpType

N, DV, M, DE, E = 2048, 32, 4096, 16, 65536
D, WD, L, H = 256, 64, 2, 4
NCORES = 8
MASK_C = 30.0
PS_B = 8   # 2KB PSUM slots (banks) in the main pool
GRP = 6    # tiles processed stage-interleaved
SBUF_B = 3 * GRP


def _sab_layer(nc, pools, items, zb, ident, negc, ones1):
    """One SAB layer on a group of stage-interleaved 256-row tiles.

    items: list of (Xt, C); Xt is a [128, 2, 256] bf16 tile (feature-major,
    fb chunks).  Returns the new (Xt, C) list.
    """
    sb, ps = pools
    n = len(items)

    # --- Q/K projections (feature-major) ---
    psQ, psK = [], []
    for i, (Xt, C) in enumerate(items):
        W = C["W"]
        q = ps.tile([128, 2, 256], F32, tag="bank", name="psQ", bufs=PS_B)
        k = ps.tile([128, 2, 256], F32, tag="bank", name="psK", bufs=PS_B)
        for fb in range(2):
            for kb in range(2):
                nc.tensor.matmul(q[:, fb, :], W("q", kb, fb),
                                 Xt[:, kb, :], start=(kb == 0), stop=(kb == 1))
                nc.tensor.matmul(k[:, fb, :], W("k", kb, fb),
                                 Xt[:, kb, :], start=(kb == 0), stop=(kb == 1))
        psQ.append(q)
        psK.append(k)
    Qt = [sb.tile([128, 2, 256], BF16, tag="Qt", name="Qt", bufs=SBUF_B)
          for _ in range(n)]
    Kt = [sb.tile([128, 2, 256], BF16, tag="Kt", name="Kt", bufs=SBUF_B)
          for _ in range(n)]
    for i, (Xt, C) in enumerate(items):
        if zb:
            (nc.vector.tensor_copy if i % 2 else nc.scalar.copy)(Qt[i][:], psQ[i][:])
        else:
            Bq = C["Bq"]
            for fb in range(2):
                nc.vector.tensor_scalar_add(Qt[i][:, fb, :], psQ[i][:, fb, :],
                                            Bq[:, fb:fb + 1])
        (nc.scalar.copy if i % 2 else nc.vector.tensor_copy)(Kt[i][:], psK[i][:])

    # --- V projection (row-major) ---
    psV = []
    for i, (Xt, C) in enumerate(items):
        W = C["W"]
        v = ps.tile([128, 2, 4, 64], F32, tag="bank", name="psV", bufs=PS_B)
        for rb in range(2):
            for kb in range(2):
                nc.tensor.matmul(v[:, rb, :, :],
                                 Xt[:, kb, rb * 128:(rb + 1) * 128],
                                 W("v", kb, None), start=(kb == 0), stop=(kb == 1))
        psV.append(v)
    Vr = [sb.tile([128, 2, 4, 64], BF16, tag="Vr", name="Vr", bufs=2 * GRP)
          for _ in range(n)]
    for i in range(n):
        (nc.vector.tensor_copy if i % 2 else nc.scalar.copy)(Vr[i][:], psV[i][:])

    # --- attention per diagonal 128x128 block ---
    O_rm = [sb.tile([128, 2, 4, 64], BF16, tag="Orm", name="O_rm",
                    bufs=2 * GRP) for _ in range(n)]
    for b in range(2):
        psS = []
        for i, (Xt, C) in enumerate(items):
            s = ps.tile([128, 512], F32, tag="bank", name="psS", bufs=PS_B)
            for h in range(H):
                p, off = h // 2, (h % 2) * 64
                bs = slice(b * 128, (b + 1) * 128)
                nc.tensor.matmul(s[:, h * 128:(h + 1) * 128],
                                 C["Gk"][:], C["Gq"][:], start=True, stop=False)
                nc.tensor.matmul(s[:, h * 128:(h + 1) * 128],
                                 Kt[i][off:off + 64, p, bs],
                                 Qt[i][off:off + 64, p, bs],
                                 start=False, stop=True)
            psS.append(s)
        eS = []
        for i in range(n):
            e = sb.tile([128, 512], BF16, tag="eS", name="eS", bufs=2 * GRP)
            nc.scalar.activation(e[:], psS[i][:], AF.Exp, bias=negc[:],
                                 scale=1.0 / 16.0)
            eS.append(e)
        for i in range(n):
            pO = ps.tile([128, 4, 65], F32, tag="bank", name="pO", bufs=PS_B)
            for h in range(H):
                nc.tensor.matmul(pO[:, h, 0:64], eS[i][:, h * 128:(h + 1) * 128],
                                 Vr[i][:, b, h, :], start=True, stop=True)
                nc.tensor.matmul(pO[:, h, 64:65],
                                 eS[i][:, h * 128:(h + 1) * 128],
                                 ones1[:], start=True, stop=True)
            rec = sb.tile([128, 4, 1], F32, tag="rec", name="rec", bufs=2 * GRP)
            nc.vector.reciprocal(rec[:], pO[:, :, 64:65])
            nc.vector.tensor_tensor(O_rm[i][:, b, :, :], pO[:, :, 0:64],
                                    rec[:].broadcast_to([128, 4, 64]), ALU.mult)

    # --- transpose O back to feature-major, + Qh ---
    Ot = [sb.tile([128, 2, 256], BF16, tag="Ot", name="Ot", bufs=SBUF_B)
          for _ in range(n)]
    for i, (Xt, C) in enumerate(items):
        psOT = ps.tile([128, 2, 256], BF16, tag="bank", name="psOT", bufs=PS_B)
        for b in range(2):
            for fb in range(2):
                nc.tensor.transpose(psOT[:, fb, b * 128:(b + 1) * 128],
                                    O_rm[i][:, b, fb * 2:fb * 2 + 2, :],
                                    ident[:])
        if zb:
            nc.vector.tensor_add(Ot[i][:], psOT[:], Qt[i][:])
        else:
            Bv = C["Bv"]
            for fb in range(2):
                nc.vector.scalar_tensor_tensor(
                    Ot[i][:, fb, :], psOT[:, fb, :],
                    Bv[:, fb:fb + 1], Qt[i][:, fb, :], ALU.add, ALU.add)

    # --- Z = O + relu(O @ Wo + bo) ---
    psR = []
    for i, (Xt, C) in enumerate(items):
        W = C["W"]
        r = ps.tile([128, 2, 256], F32, tag="bank", name="psR", bufs=PS_B)
        for fb in range(2):
            for kb in range(2):
                nc.tensor.matmul(r[:, fb, :], W("o", kb, fb),
                                 Ot[i][:, kb, :], start=(kb == 0), stop=(kb == 1))
        psR.append(r)
    out = []
    for i, (Xt, C) in enumerate(items):
        Rt = sb.tile([128, 2, 256], BF16, tag="Rt", name="Rt", bufs=2 * GRP)
        if zb:
            if i % 2:
                nc.vector.tensor_scalar_max(Rt[:], psR[i][:], 0.0)
            else:
                nc.scalar.activation(Rt[:], psR[i][:], AF.Relu)
        else:
            Bo = C["Bo"]
            for fb in range(2):
                nc.scalar.activation(Rt[:, fb, :], psR[i][:, fb, :],
                                     AF.Relu, bias=Bo[:, fb:fb + 1])
        Zt = sb.tile([128, 2, 256], BF16, tag="Zt", name="Zt", bufs=SBUF_B)
        nc.gpsimd.tensor_add(Zt[:], Ot[i][:], Rt[:])
        out.append((Zt, C))
    return out


def build_program(R, zb):
    """Per-core SPMD program; R = rows per core; zb = all biases zero."""
    NT = R // 256
    nc = bacc.Bacc("TRN2", target_bir_lowering=False, debug=False)

    def din(name, shape, dt=BF16):
        return nc.dram_tensor(name, shape, dt, kind="ExternalInput").ap()

    xvt_d = din("xvt", [128, 2, R])
    wvt_d = din("wvt", [WD, R])
    xet_d = din("xet", [128, 2, R])
    wet_d = din("wet", [WD, R])
    x0t_d = din("x0t", [128, 2, R])
    peW_v_d = din("peW_v", [WD, D])
    peW_e_d = din("peW_e", [WD, D])
    Wv_d = din("W_v", [128, L, 4, 2, 256])
    We_d = din("W_e", [128, L, 4, 2, 256])
    Wu_d = din("W_upd", [128, 4, 2, 256])
    # bias blob [128, 32] f32: cols 0..3 Bq_v(l,fb), 4..7 Bo_v, 8..11 Bv_v,
    # 12..23 same for e, 24..25 peb_v, 26..27 peb_e, 28..29 updb
    bias_d = din("biases", [128, 32], F32)
    Gk_v_d = din("Gk_v", [4, 128])
    Gq_v_d = din("Gq_v", [4, 128])
    Gk_e_d = din("Gk_e", [8, 128])
    Gq_e_d = din("Gq_e", [8, 128])
    ident_d = din("ident", [128, 128])

    At_d = nc.dram_tensor("At", [128, 2, R], BF16, kind="ExternalOutput").ap()
    P3t_d = nc.dram_tensor("P3t", [128, 2, R], BF16, kind="ExternalOutput").ap()

    with tile.TileContext(nc) as tc, ExitStack() as es, \
            nc.allow_low_precision(reason="bf16 activations, fp32 PSUM accum"):
        const = es.enter_context(tc.tile_pool(name="const", bufs=1))
        sb = es.enter_context(tc.tile_pool(name="sb", bufs=4))
        inp = es.enter_context(tc.tile_pool(name="inp", bufs=3 * GRP))
        outp = es.enter_context(tc.tile_pool(name="outp", bufs=2 * GRP))
        ps = es.enter_context(tc.tile_pool(name="ps", bufs=1, space="PSUM"))
        pools = (sb, ps)

        negc = const.tile([128, 1], F32, tag="negc", name="negc")
        nc.vector.memset(negc[:], -MASK_C)
        ones1 = const.tile([128, 1], BF16, tag="ones1", name="ones1")
        nc.vector.memset(ones1[:], 1.0)
        ident = const.tile([128, 128], BF16, tag="ident", name="ident")
        nc.sync.dma_start(ident[:], ident_d)

        bias = const.tile([128, 32], F32, tag="bias", name="bias")
        nc.sync.dma_start(bias[:], bias_d)

        peW = {}
        for s, peW_d in (("v", peW_v_d), ("e", peW_e_d)):
            t = const.tile([WD, D], BF16, tag=f"peW_{s}", name=f"peW_{s}")
            nc.sync.dma_start(t[:], peW_d)
            peW[s] = t
        peb = {"v": bias[:, 24:26], "e": bias[:, 26:28]}
        updb = bias[:, 28:30]

        Wall, G = {}, {}
        for s, W_d in (("v", Wv_d), ("e", We_d)):
            t = const.tile([128, L, 4, 2, 256], BF16, tag=f"W_{s}", name=f"W_{s}")
            nc.sync.dma_start(t[:], W_d)
            Wall[s] = t
        for s, gk_d, gq_d, g in (("v", Gk_v_d, Gq_v_d, 4), ("e", Gk_e_d, Gq_e_d, 8)):
            gk = const.tile([g, 128], BF16, tag=f"Gk{s}", name=f"Gk{s}")
            nc.sync.dma_start(gk[:], gk_d)
            gq = const.tile([g, 128], BF16, tag=f"Gq{s}", name=f"Gq{s}")
            nc.sync.dma_start(gq[:], gq_d)
            G[s] = (gk, gq)
        Wu = const.tile([128, 4, 2, 256], BF16, tag="Wu", name="Wu")
        nc.sync.dma_start(Wu[:], Wu_d)

        PI = {"q": 0, "k": 1, "v": 2, "o": 3}

        def mkW(s, l):
            def W(p, kb, fb):
                if fb is None:
                    return Wall[s][:, l, PI[p], kb, :]
                return Wall[s][:, l, PI[p], kb, fb * 128:(fb + 1) * 128]
            return W

        side_consts = {}
        for s in ("v", "e"):
            off = 0 if s == "v" else 12
            side_consts[s] = [{
                "W": mkW(s, l),
                "Bq": bias[:, off + l * 2:off + l * 2 + 2],
                "Bo": bias[:, off + 4 + l * 2:off + 4 + l * 2 + 2],
                "Bv": bias[:, off + 8 + l * 2:off + 8 + l * 2 + 2],
                "Gk": G[s][0], "Gq": G[s][1],
            } for l in range(L)]

        side_io = {"v": (xvt_d, wvt_d, At_d), "e": (xet_d, wet_d, P3t_d)}
        tiles = [(s, t) for t in range(NT) for s in ("v", "e")]
        groups = [tiles[g:g + GRP] for g in range(0, len(tiles), GRP)]

        for grp in groups:
            # --- loads ---
            xts, wts, x0s = [], [], {}
            for s, t in grp:
                cs = slice(t * 256, (t + 1) * 256)
                xt_d, wt_d, _ = side_io[s]
                xt = inp.tile([128, 2, 256], BF16, tag="xt", name="xt")
                nc.sync.dma_start(xt[:], xt_d[:, :, cs])
                xts.append(xt)
                wt = inp.tile([WD, 256], BF16, tag="wt", name="wt")
                nc.sync.dma_start(wt[:], wt_d[:, cs])
                wts.append(wt)
                if s == "v" and t not in x0s:
                    x0 = inp.tile([128, 2, 256], BF16, tag="x0", name="x0",
                                  bufs=GRP)
                    nc.sync.dma_start(x0[:], x0t_d[:, :, cs])
                    x0s[t] = x0

            # --- mailbox prep: Xt = xt + peW^T wt (+ peb) ---
            psP, Xts = [], []
            for gi, (s, t) in enumerate(grp):
                p = ps.tile([128, 2, 256], F32, tag="bank", name="psP",
                            bufs=PS_B)
                for fb in range(2):
                    nc.tensor.matmul(p[:, fb, :],
                                     peW[s][:, fb * 128:(fb + 1) * 128],
                                     wts[gi][:], start=True, stop=True)
                psP.append(p)
            for gi, (s, t) in enumerate(grp):
                Xt = sb.tile([128, 2, 256], BF16, tag="Xt", name="Xt",
                             bufs=SBUF_B)
                if zb:
                    nc.vector.tensor_add(Xt[:], psP[gi][:], xts[gi][:])
                else:
                    for fb in range(2):
                        nc.vector.scalar_tensor_tensor(
                            Xt[:, fb, :], psP[gi][:, fb, :],
                            peb[s][:, fb:fb + 1], xts[gi][:, fb, :],
                            ALU.add, ALU.add)
                Xts.append(Xt)

            # --- 2 SAB layers, stage-interleaved across the group ---
            items = [(Xts[gi], side_consts[s][0]) for gi, (s, t) in enumerate(grp)]
            items = _sab_layer(nc, pools, items, zb, ident, negc, ones1)
            items = [(items[gi][0], side_consts[s][1])
                     for gi, (s, t) in enumerate(grp)]
            items = _sab_layer(nc, pools, items, zb, ident, negc, ones1)

            # --- update linear, feature-major: out[of, rows] ---
            psA = []
            for gi, (s, t) in enumerate(grp):
                Zt = items[gi][0]
                a = ps.tile([128, 2, 256], F32, tag="bank", name="psA",
                            bufs=PS_B)
                srcs = (((xts[gi], 0), (Zt, 1), (x0s[t], 3)) if s == "v"
                        else ((Zt, 2),))
                for of in range(2):
                    first = True
                    for src, j in srcs:
                        for kb in range(2):
                            nc.tensor.matmul(
                                a[:, of, :],
                                Wu[:, j, kb, of * 128:(of + 1) * 128],
                                src[:, kb, :], start=first,
                                stop=(src is srcs[-1][0] and kb == 1))
                            first = False
                psA.append(a)
            for gi, (s, t) in enumerate(grp):
                cs = slice(t * 256, (t + 1) * 256)
                out_d = side_io[s][2]
                Ao = outp.tile([128, 2, 256], BF16, tag="Ao", name="Ao")
                if zb or s == "e":
                    (nc.scalar.copy if gi % 2 else nc.vector.tensor_copy)(Ao[:], psA[gi][:])
                else:
                    for of in range(2):
                        nc.scalar.activation(Ao[:, of, :], psA[gi][:, of, :],
                                             AF.Identity,
                                             bias=updb[:, of:of + 1])
                nc.sync.dma_start(out_d[:, :, cs], Ao[:])

    nc.compile()
    return nc


def _make_group_consts(n_group):
    """Gk [G,128] (bf16, 16C on own group) and Gq4 [G,512] (1 on own group,
    tiled for 4 heads)."""
    G = 128 // n_group
    Gk = np.zeros((G, 128), np.float32)
    Gq = np.zeros((G, 128), np.float32)
    for g in range(G):
        Gk[g, g * n_group:(g + 1) * n_group] = 16.0 * MASK_C
        Gq[g, g * n_group:(g + 1) * n_group] = 1.0
    return Gk, Gq


_PROGRAM_CACHE = {}


def _get_program(R, zb=True):
    key = (R, zb)
    if key not in _PROGRAM_CACHE:
        _PROGRAM_CACHE[key] = build_program(R, zb)
    return _PROGRAM_CACHE[key]


def kernel(co_feat_in, co_feat_con, co_feat_0, weight_in, weight_con,
           pe_v_W, pe_v_b, pe_e_W, pe_e_b,
           Wq_v, bq_v, Wk_v, bk_v, Wv_v, bv_v, Wo_v, bo_v,
           Wq_e, bq_e, Wk_e, bk_e, Wv_e, bv_e, Wo_e, bo_e,
           upd_W, upd_b, perm):
    bf = np.dtype(mybir.dt.np(BF16))
    f = lambda x: np.asarray(x, np.float32)
    perm = np.asarray(perm)

    R = E // NCORES
    zb = all(not np.any(f(b)) for b in
             (bq_v, bv_v, bo_v, bq_e, bv_e, bo_e, pe_v_b, pe_e_b, upd_b))
    nc = _get_program(R, zb)

    Gk_v, Gq_v = _make_group_consts(DV)
    Gk_e, Gq_e = _make_group_consts(DE)

    # bias blob [128, 32]
    bias = np.zeros((128, 32), np.float32)
    for off, (bq, bo, bv) in ((0, (bq_v, bo_v, bv_v)), (12, (bq_e, bo_e, bv_e))):
        for l in range(L):
            for fb in range(2):
                bias[:, off + l * 2 + fb] = f(bq)[l, fb * 128:(fb + 1) * 128]
                bias[:, off + 4 + l * 2 + fb] = f(bo)[l, fb * 128:(fb + 1) * 128]
                bias[:, off + 8 + l * 2 + fb] = f(bv)[l, fb * 128:(fb + 1) * 128]
    for fb in range(2):
        bias[:, 24 + fb] = f(pe_v_b)[fb * 128:(fb + 1) * 128]
        bias[:, 26 + fb] = f(pe_e_b)[fb * 128:(fb + 1) * 128]
        bias[:, 28 + fb] = f(upd_b)[fb * 128:(fb + 1) * 128]

    def wblob(Wq, Wk, Wv, Wo):
        # [L,4,D,D] -> [128, L, 4, 2, 256]
        Ws = np.stack([f(Wq), f(Wk), f(Wv), f(Wo)], axis=1)  # [L,4,D,D]
        return np.ascontiguousarray(
            Ws.reshape(L, 4, 2, 128, 256).transpose(3, 0, 1, 2, 4)).astype(bf)

    Wu = np.ascontiguousarray(
        f(upd_W).reshape(4, 2, 128, 256).transpose(2, 0, 1, 3)).astype(bf)

    def xlayout(x, rs):
        # [R, 256] slice -> [128, 2, R]
        a = np.asarray(x)[rs]
        return np.ascontiguousarray(
            a.reshape(a.shape[0], 2, 128).transpose(2, 1, 0)).astype(bf)

    shared = {
        "peW_v": f(pe_v_W).astype(bf), "peW_e": f(pe_e_W).astype(bf),
        "W_v": wblob(Wq_v, Wk_v, Wv_v, Wo_v),
        "W_e": wblob(Wq_e, Wk_e, Wv_e, Wo_e),
        "W_upd": Wu,
        "biases": bias,
        "Gk_v": Gk_v.astype(bf), "Gq_v": Gq_v.astype(bf),
        "Gk_e": Gk_e.astype(bf), "Gq_e": Gq_e.astype(bf),
        "ident": np.eye(128, dtype=np.float32).astype(bf),
    }

    in_maps = []
    for c in range(NCORES):
        rs = slice(c * R, (c + 1) * R)
        m = dict(shared)
        m["xvt"] = xlayout(co_feat_in, rs)
        m["xet"] = xlayout(co_feat_con, rs)
        m["x0t"] = xlayout(co_feat_0, rs)
        m["wvt"] = np.asarray(weight_in)[rs].T.astype(bf)
        m["wet"] = np.asarray(weight_con)[rs].T.astype(bf)
        in_maps.append(m)

    res = run_bass_kernel_spmd(nc, in_maps, core_ids=list(range(NCORES)))

    def unlayout(o):
        # [128, 2, R] -> [R, 256] f32
        return np.asarray(o).transpose(2, 1, 0).reshape(R, 256).astype(np.float32)

    A = np.concatenate([unlayout(res.results[c]["At"]) for c in range(NCORES)])
    P3 = np.concatenate([unlayout(res.results[c]["P3t"]) for c in range(NCORES)])

    inv_perm = np.argsort(perm)
    out_in = A + P3[inv_perm]
    return np.stack([out_in, out_in[perm]]).astype(np.float32)


# revision 16
# speedup vs baseline: 1.4744x; 1.1009x over previous
"""CoNHD GD-layer Trainium2 kernel — skewed-pipeline variant (v7).

Same math/layout as v6 (see kernel.py docstring), but tiles advance
through a ~20-stage pipeline with staggered starts: at any wall-clock
moment the live tiles sit at different stages, so every engine's
in-order queue mixes projection, evacuation, exp and update work and
cross-engine waits of one tile hide behind another tile's compute.
"""
import sys

if "/opt/trn_rl_repo" not in sys.path:
    sys.path.insert(0, "/opt/trn_rl_repo")

from contextlib import ExitStack
from types import SimpleNamespace

import numpy as np

import concourse.mybir as mybir
import concourse.tile as tile
from concourse import bacc
from concourse.bass_utils import run_bass_kernel_spmd

F32 = mybir.dt.float32
BF16 = mybir.dt.bfloat16
AF = mybir.ActivationFunctionType
ALU = mybir.AluOpType

N, DV, M, DE, E = 2048, 32, 4096, 16, 65536
D, WD, L, H = 256, 64, 2, 4
NCORES = 8
MASK_C = 30.0
PS_B = 8       # 2KB PSUM slots (banks), one shared ring
START_GAP = 2  # waves between tile starts
SBUF_B = 8     # per-tag SBUF ring depth


def _tile_stages(nc, pools, env, side, t, idx, zb):
    """Build the stage-callable list for one 256-row tile."""
    sb, ps, inp, outp = pools
    st = SimpleNamespace()
    C = env["side_consts"][side]
    xt_d, wt_d, out_d = env["side_io"][side]
    x0t_d = env["x0t_d"]
    peW, peb = env["peW"][side], env["peb"][side]
    Wu, updb = env["Wu"], env["updb"]
    ident, negc, ones1 = env["ident"], env["negc"], env["ones1"]
    cs = slice(t * 256, (t + 1) * 256)
    # engines alternate per tile to split each phase's evac load
    ev0 = nc.vector.tensor_copy if idx % 2 else nc.scalar.copy
    ev1 = nc.scalar.copy if idx % 2 else nc.vector.tensor_copy

    def load():
        st.xt = inp.tile([128, 2, 256], BF16, tag="xt", name="xt", bufs=SBUF_B)
        nc.sync.dma_start(st.xt[:], xt_d[:, :, cs])
        st.wt = inp.tile([WD, 256], BF16, tag="wt", name="wt", bufs=SBUF_B)
        nc.sync.dma_start(st.wt[:], wt_d[:, cs])
        if side == "v":
            st.x0 = inp.tile([128, 2, 256], BF16, tag="x0", name="x0",
                             bufs=SBUF_B // 2)
            nc.sync.dma_start(st.x0[:], x0t_d[:, :, cs])

    def prep():
        psP = ps.tile([128, 2, 256], F32, tag="bank", name="psP", bufs=PS_B)
        for fb in range(2):
            nc.tensor.matmul(psP[:, fb, :], peW[:, fb * 128:(fb + 1) * 128],
                             st.wt[:], start=True, stop=True)
        st.Xt = sb.tile([128, 2, 256], BF16, tag="Xt", name="Xt", bufs=SBUF_B)
        if zb:
            nc.vector.tensor_add(st.Xt[:], psP[:], st.xt[:])
        else:
            for fb in range(2):
                nc.vector.scalar_tensor_tensor(
                    st.Xt[:, fb, :], psP[:, fb, :], peb[:, fb:fb + 1],
                    st.xt[:, fb, :], ALU.add, ALU.add)

    stages = [load, prep]

    def mk_layer(l):
        Cl = C[l]
        W = Cl["W"]

        def qk_mm():
            st.psQ = ps.tile([128, 2, 256], F32, tag="bank", name="psQ",
                             bufs=PS_B)
            st.psK = ps.tile([128, 2, 256], F32, tag="bank", name="psK",
                             bufs=PS_B)
            for fb in range(2):
                for kb in range(2):
                    nc.tensor.matmul(st.psQ[:, fb, :], W("q", kb, fb),
                                     st.Xt[:, kb, :], start=(kb == 0),
                                     stop=(kb == 1))
                    nc.tensor.matmul(st.psK[:, fb, :], W("k", kb, fb),
                                     st.Xt[:, kb, :], start=(kb == 0),
                                     stop=(kb == 1))

        def qk_evac():
            st.Qt = sb.tile([128, 2, 256], BF16, tag="Qt", name="Qt",
                            bufs=SBUF_B)
            st.Kt = sb.tile([128, 2, 256], BF16, tag="Kt", name="Kt",
                            bufs=SBUF_B)
            if zb:
                ev0(st.Qt[:], st.psQ[:])
            else:
                for fb in range(2):
                    nc.vector.tensor_scalar_add(st.Qt[:, fb, :],
                                                st.psQ[:, fb, :],
                                                Cl["Bq"][:, fb:fb + 1])
            ev1(st.Kt[:], st.psK[:])

        def v_mm():
            psV = ps.tile([128, 2, 4, 64], F32, tag="bank", name="psV",
                          bufs=PS_B)
            for rb in range(2):
                for kb in range(2):
                    nc.tensor.matmul(psV[:, rb, :, :],
                                     st.Xt[:, kb, rb * 128:(rb + 1) * 128],
                                     W("v", kb, None), start=(kb == 0),
                                     stop=(kb == 1))
            st.Vr = sb.tile([128, 2, 4, 64], BF16, tag="Vr", name="Vr",
                            bufs=SBUF_B)
            ev0(st.Vr[:], psV[:])
            st.O_rm = sb.tile([128, 2, 4, 64], BF16, tag="Orm", name="O_rm",
                              bufs=SBUF_B)

        def mk_score(b):
            def score():
                st.psS = ps.tile([128, 512], F32, tag="bank", name="psS",
                                 bufs=PS_B)
                for h in range(H):
                    p, off = h // 2, (h % 2) * 64
                    bs = slice(b * 128, (b + 1) * 128)
                    nc.tensor.matmul(st.psS[:, h * 128:(h + 1) * 128],
                                     Cl["Gk"][:], Cl["Gq"][:],
                                     start=True, stop=False)
                    nc.tensor.matmul(st.psS[:, h * 128:(h + 1) * 128],
                                     st.Kt[off:off + 64, p, bs],
                                     st.Qt[off:off + 64, p, bs],
                                     start=False, stop=True)
                st.eS = sb.tile([128, 512], BF16, tag="eS", name="eS",
                                bufs=SBUF_B)
                nc.scalar.activation(st.eS[:], st.psS[:], AF.Exp,
                                     bias=negc[:], scale=1.0 / 16.0)
            return score

        def mk_av(b):
            def av():
                pO = ps.tile([128, 4, 65], F32, tag="bank", name="pO",
                             bufs=PS_B)
                for h in range(H):
                    nc.tensor.matmul(pO[:, h, 0:64],
                                     st.eS[:, h * 128:(h + 1) * 128],
                                     st.Vr[:, b, h, :], start=True, stop=True)
                    nc.tensor.matmul(pO[:, h, 64:65],
                                     st.eS[:, h * 128:(h + 1) * 128],
                                     ones1[:], start=True, stop=True)
                rec = sb.tile([128, 4, 1], F32, tag="rec", name="rec",
                              bufs=SBUF_B)
                nc.vector.reciprocal(rec[:], pO[:, :, 64:65])
                nc.vector.tensor_tensor(st.O_rm[:, b, :, :], pO[:, :, 0:64],
                                        rec[:].broadcast_to([128, 4, 64]),
                                        ALU.mult)
            return av

        def transp():
            psOT = ps.tile([128, 2, 256], BF16, tag="bank", name="psOT",
                           bufs=PS_B)
            for b in range(2):
                for fb in range(2):
                    nc.tensor.transpose(psOT[:, fb, b * 128:(b + 1) * 128],
                                        st.O_rm[:, b, fb * 2:fb * 2 + 2, :],
                                        ident[:])
            st.Ot = sb.tile([128, 2, 256], BF16, tag="Ot", name="Ot",
                            bufs=SBUF_B)
            if zb:
                nc.vector.tensor_add(st.Ot[:], psOT[:], st.Qt[:])
            else:
                for fb in range(2):
                    nc.vector.scalar_tensor_tensor(
                        st.Ot[:, fb, :], psOT[:, fb, :],
                        Cl["Bv"][:, fb:fb + 1], st.Qt[:, fb, :],
                        ALU.add, ALU.add)

        def wo():
            psR = ps.tile([128, 2, 256], F32, tag="bank", name="psR",
                          bufs=PS_B)
            for fb in range(2):
                for kb in range(2):
                    nc.tensor.matmul(psR[:, fb, :], W("o", kb, fb),
                                     st.Ot[:, kb, :], start=(kb == 0),
                                     stop=(kb == 1))
            Rt = sb.tile([128, 2, 256], BF16, tag="Rt", name="Rt",
                         bufs=SBUF_B)
            if zb:
                if idx % 2:
                    nc.vector.tensor_scalar_max(Rt[:], psR[:], 0.0)
                else:
                    nc.scalar.activation(Rt[:], psR[:], AF.Relu)
            else:
                for fb in range(2):
                    nc.scalar.activation(Rt[:, fb, :], psR[:, fb, :],
                                         AF.Relu, bias=Cl["Bo"][:, fb:fb + 1])
            st.Xt = sb.tile([128, 2, 256], BF16, tag="Xt", name="Xt",
                            bufs=SBUF_B)
            nc.gpsimd.tensor_add(st.Xt[:], st.Ot[:], Rt[:])

        return [qk_mm, qk_evac, v_mm, mk_score(0), mk_av(0), mk_score(1),
                mk_av(1), transp, wo]

    for l in range(L):
        stages.extend(mk_layer(l))

    def upd():
        st.psA = ps.tile([128, 2, 256], F32, tag="bank", name="psA",
                         bufs=PS_B)
        srcs = (((st.xt, 0), (st.Xt, 1), (st.x0, 3)) if side == "v"
                else ((st.Xt, 2),))
        for of in range(2):
            first = True
            for src, j in srcs:
                for kb in range(2):
                    nc.tensor.matmul(
                        st.psA[:, of, :],
                        Wu[:, j, kb, of * 128:(of + 1) * 128],
                        src[:, kb, :], start=first,
                        stop=(src is srcs[-1][0] and kb == 1))
                    first = False

    def out():
        Ao = outp.tile([128, 2, 256], BF16, tag="Ao", name="Ao",
                       bufs=SBUF_B // 2)
        if zb or side == "e":
            ev0(Ao[:], st.psA[:])
        else:
            for of in range(2):
                nc.scalar.activation(Ao[:, of, :], st.psA[:, of, :],
                                     AF.Identity, bias=updb[:, of:of + 1])
        nc.sync.dma_start(out_d[:, :, cs], Ao[:])

    stages.extend([upd, out])
    return stages


def build_program(R, zb):
    """Per-core SPMD program; R = rows per core; zb = all biases zero."""
    NT = R // 256
    nc = bacc.Bacc("TRN2", target_bir_lowering=False, debug=False)

    def din(name, shape, dt=BF16):
        return nc.dram_tensor(name, shape, dt, kind="ExternalInput").ap()

    xvt_d = din("xvt", [128, 2, R])
    wvt_d = din("wvt", [WD, R])
    xet_d = din("xet", [128, 2, R])
    wet_d = din("wet", [WD, R])
    x0t_d = din("x0t", [128, 2, R])
    peW_v_d = din("peW_v", [WD, D])
    peW_e_d = din("peW_e", [WD, D])
    Wv_d = din("W_v", [128, L, 4, 2, 256])
    We_d = din("W_e", [128, L, 4, 2, 256])
    Wu_d = din("W_upd", [128, 4, 2, 256])
    bias_d = din("biases", [128, 32], F32)
    Gk_v_d = din("Gk_v", [4, 128])
    Gq_v_d = din("Gq_v", [4, 128])
    Gk_e_d = din("Gk_e", [8, 128])
    Gq_e_d = din("Gq_e", [8, 128])
    ident_d = din("ident", [128, 128])

    At_d = nc.dram_tensor("At", [128, 2, R], BF16, kind="ExternalOutput").ap()
    P3t_d = nc.dram_tensor("P3t", [128, 2, R], BF16, kind="ExternalOutput").ap()

    with tile.TileContext(nc) as tc, ExitStack() as es, \
            nc.allow_low_precision(reason="bf16 activations, fp32 PSUM accum"):
        const = es.enter_context(tc.tile_pool(name="const", bufs=1))
        sb = es.enter_context(tc.tile_pool(name="sb", bufs=4))
        inp = es.enter_context(tc.tile_pool(name="inp", bufs=SBUF_B))
        outp = es.enter_context(tc.tile_pool(name="outp", bufs=4))
        ps = es.enter_context(tc.tile_pool(name="ps", bufs=1, space="PSUM"))
        pools = (sb, ps, inp, outp)

        negc = const.tile([128, 1], F32, tag="negc", name="negc")
        nc.vector.memset(negc[:], -MASK_C)
        ones1 = const.tile([128, 1], BF16, tag="ones1", name="ones1")
        nc.vector.memset(ones1[:], 1.0)
        ident = const.tile([128, 128], BF16, tag="ident", name="ident")
        nc.sync.dma_start(ident[:], ident_d)
        bias = const.tile([128, 32], F32, tag="bias", name="bias")
        nc.sync.dma_start(bias[:], bias_d)

        peW = {}
        for s, peW_d in (("v", peW_v_d), ("e", peW_e_d)):
            tl = const.tile([WD, D], BF16, tag=f"peW_{s}", name=f"peW_{s}")
            nc.sync.dma_start(tl[:], peW_d)
            peW[s] = tl
        peb = {"v": bias[:, 24:26], "e": bias[:, 26:28]}
        updb = bias[:, 28:30]

        Wall, G = {}, {}
        for s, W_d in (("v", Wv_d), ("e", We_d)):
            tl = const.tile([128, L, 4, 2, 256], BF16, tag=f"W_{s}",
                            name=f"W_{s}")
            nc.sync.dma_start(tl[:], W_d)
            Wall[s] = tl
        for s, gk_d, gq_d, g in (("v", Gk_v_d, Gq_v_d, 4),
                                 ("e", Gk_e_d, Gq_e_d, 8)):
            gk = const.tile([g, 128], BF16, tag=f"Gk{s}", name=f"Gk{s}")
            nc.sync.dma_start(gk[:], gk_d)
            gq = const.tile([g, 128], BF16, tag=f"Gq{s}", name=f"Gq{s}")
            nc.sync.dma_start(gq[:], gq_d)
            G[s] = (gk, gq)
        Wu = const.tile([128, 4, 2, 256], BF16, tag="Wu", name="Wu")
        nc.sync.dma_start(Wu[:], Wu_d)

        PI = {"q": 0, "k": 1, "v": 2, "o": 3}

        def mkW(s, l):
            def W(p, kb, fb):
                if fb is None:
                    return Wall[s][:, l, PI[p], kb, :]
                return Wall[s][:, l, PI[p], kb, fb * 128:(fb + 1) * 128]
            return W

        side_consts = {}
        for s in ("v", "e"):
            off = 0 if s == "v" else 12
            side_consts[s] = [{
                "W": mkW(s, l),
                "Bq": bias[:, off + l * 2:off + l * 2 + 2],
                "Bo": bias[:, off + 4 + l * 2:off + 4 + l * 2 + 2],
                "Bv": bias[:, off + 8 + l * 2:off + 8 + l * 2 + 2],
                "Gk": G[s][0], "Gq": G[s][1],
            } for l in range(L)]

        env = {
            "side_consts": side_consts,
            "side_io": {"v": (xvt_d, wvt_d, At_d), "e": (xet_d, wet_d, P3t_d)},
            "x0t_d": x0t_d, "peW": peW, "peb": peb, "Wu": Wu, "updb": updb,
            "ident": ident, "negc": negc, "ones1": ones1,
        }

        descs = [(s, t) for t in range(NT) for s in ("v", "e")]
        live = []  # list of stage-iterators
        di = 0
        wave = 0
        while di < len(descs) or live:
            if di < len(descs) and wave % START_GAP == 0:
                s, t = descs[di]
                live.append(iter(_tile_stages(nc, pools, env, s, t, di, zb)))
                di += 1
            for it in list(live):
                stage = next(it, None)
                if stage is None:
                    live.remove(it)
                else:
                    stage()
            wave += 1

    nc.compile()
    return nc


def _make_group_consts(n_group):
    G = 128 // n_group
    Gk = np.zeros((G, 128), np.float32)
    Gq = np.zeros((G, 128), np.float32)
    for g in range(G):
        Gk[g, g * n_group:(g + 1) * n_group] = 16.0 * MASK_C
        Gq[g, g * n_group:(g + 1) * n_group] = 1.0
    return Gk, Gq


_PROGRAM_CACHE = {}


def _get_program(R, zb=True):
    key = (R, zb)
    if key not in _PROGRAM_CACHE:
        _PROGRAM_CACHE[key] = build_program(R, zb)
    return _PROGRAM_CACHE[key]


def kernel(co_feat_in, co_feat_con, co_feat_0, weight_in, weight_con,
           pe_v_W, pe_v_b, pe_e_W, pe_e_b,
           Wq_v, bq_v, Wk_v, bk_v, Wv_v, bv_v, Wo_v, bo_v,
           Wq_e, bq_e, Wk_e, bk_e, Wv_e, bv_e, Wo_e, bo_e,
           upd_W, upd_b, perm):
    bf = np.dtype(mybir.dt.np(BF16))
    f = lambda x: np.asarray(x, np.float32)
    perm = np.asarray(perm)

    R = E // NCORES
    zb = all(not np.any(f(b)) for b in
             (bq_v, bv_v, bo_v, bq_e, bv_e, bo_e, pe_v_b, pe_e_b, upd_b))
    nc = _get_program(R, zb)

    Gk_v, Gq_v = _make_group_consts(DV)
    Gk_e, Gq_e = _make_group_consts(DE)

    bias = np.zeros((128, 32), np.float32)
    for off, (bq, bo, bv) in ((0, (bq_v, bo_v, bv_v)), (12, (bq_e, bo_e, bv_e))):
        for l in range(L):
            for fb in range(2):
                bias[:, off + l * 2 + fb] = f(bq)[l, fb * 128:(fb + 1) * 128]
                bias[:, off + 4 + l * 2 + fb] = f(bo)[l, fb * 128:(fb + 1) * 128]
                bias[:, off + 8 + l * 2 + fb] = f(bv)[l, fb * 128:(fb + 1) * 128]
    for fb in range(2):
        bias[:, 24 + fb] = f(pe_v_b)[fb * 128:(fb + 1) * 128]
        bias[:, 26 + fb] = f(pe_e_b)[fb * 128:(fb + 1) * 128]
        bias[:, 28 + fb] = f(upd_b)[fb * 128:(fb + 1) * 128]

    def wblob(Wq, Wk, Wv, Wo):
        Ws = np.stack([f(Wq), f(Wk), f(Wv), f(Wo)], axis=1)
        return np.ascontiguousarray(
            Ws.reshape(L, 4, 2, 128, 256).transpose(3, 0, 1, 2, 4)).astype(bf)

    Wu = np.ascontiguousarray(
        f(upd_W).reshape(4, 2, 128, 256).transpose(2, 0, 1, 3)).astype(bf)

    def xlayout(x, rs):
        a = np.asarray(x)[rs]
        return np.ascontiguousarray(
            a.reshape(a.shape[0], 2, 128).transpose(2, 1, 0)).astype(bf)

    shared = {
        "peW_v": f(pe_v_W).astype(bf), "peW_e": f(pe_e_W).astype(bf),
        "W_v": wblob(Wq_v, Wk_v, Wv_v, Wo_v),
        "W_e": wblob(Wq_e, Wk_e, Wv_e, Wo_e),
        "W_upd": Wu,
        "biases": bias,
        "Gk_v": Gk_v.astype(bf), "Gq_v": Gq_v.astype(bf),
        "Gk_e": Gk_e.astype(bf), "Gq_e": Gq_e.astype(bf),
        "ident": np.eye(128, dtype=np.float32).astype(bf),
    }

    in_maps = []
    for c in range(NCORES):
        rs = slice(c * R, (c + 1) * R)
        m = dict(shared)
        m["xvt"] = xlayout(co_feat_in, rs)
        m["xet"] = xlayout(co_feat_con, rs)
        m["x0t"] = xlayout(co_feat_0, rs)
        m["wvt"] = np.asarray(weight_in)[rs].T.astype(bf)
        m["wet"] = np.asarray(weight_con)[rs].T.astype(bf)
        in_maps.append(m)

    res = run_bass_kernel_spmd(nc, in_maps, core_ids=list(range(NCORES)))

    def unlayout(o):
        return np.asarray(o).transpose(2, 1, 0).reshape(R, 256).astype(np.float32)

    A = np.concatenate([unlayout(res.results[c]["At"]) for c in range(NCORES)])
    P3 = np.concatenate([unlayout(res.results[c]["P3t"]) for c in range(NCORES)])

    inv_perm = np.argsort(perm)
    out_in = A + P3[inv_perm]
    return np.stack([out_in, out_in[perm]]).astype(np.float32)
